# revision 5
# baseline (speedup 1.0000x reference)
"""CrossEntropyLoss kernel for Trainium2, SPMD over 8 NeuronCores.

reference:
    gathered = output[i, label[i]]                      # [B]
    loss = (sum_i -gathered_i + sum_i log(sum_j exp(output[i,j]) + 1e-5)) / B

The kernel computes per-row sums of exp(x) on device; the host does the
final log / gather / mean (O(B) work). The O(B*V) work is split three
ways to use every engine:

  * ACT share: columns [0, V_ACT) in batch-sharded layout (each core gets
    1024 rows), uploaded as bf16. ScalarE Exp with fused row-sum
    (accum_out). ~0.8 ns/lane-elem.
  * DVE+PE share: columns [V_ACT, 32000) in vocab-sharded TRANSPOSED
    layout (each core gets VD_C=2048 v-columns x all 8192 rows,
    uploaded as fp8-e4m3 [vd, rows] so DMA lines stay >=4KB). VectorE
    casts fp8->bf16 and computes exp via the Schraudolph bit trick
    (int16(x*A+B) bitcast to bf16, one 4x-mode tensor_scalar); TensorE
    ones-matmuls then reduce over the 128 v-partitions into PSUM
    accumulators (16 row-chunks live in 6 PSUM banks at partition slots
    0/32/64), accumulated across the 16 v-tiles.

Rel-err budget is 2e-2; bf16/fp8 quantization plus the Schraudolph exp
(~2% elementwise, mean-zero) contribute ~1e-3 after row-sum averaging.

Why this shape: the problem is HBM-bound in f32 (DMA-only == full kernel
== ~290us). bf16 halves traffic but makes ACT exp (1 elem/cycle/lane,
no 16-bit speedup - measured) the wall at ~205us. Offloading ~40% of
the exp work to DVE+PE and narrowing that share to fp8 balances
DMA ~108us / ACT ~104us / DVE ~100us / PE ~75us.
"""

import numpy as np

import concourse.bass as bass
import concourse.mybir as mybir
import concourse.tile as tile
from concourse.bass_utils import run_bass_kernel_spmd

B, V = 8192, 32000
N_CORES = 8
B_LOC = B // N_CORES  # 1024 rows per core (ACT share)
P = 128
EPS = 1e-5

VD_C = 2048             # v-columns per core in the DVE+PE share
V_ACT = V - N_CORES * VD_C  # 15616 columns in the ACT share
N_RT = B_LOC // P       # 8 row tiles per core
ACT_SUB = V_ACT // 2    # 7808: ACT instruction free-dim (2 per row tile)
N_VT = VD_C // P        # 16 v-tiles per core
RH, RC, MM = 2, 8, 512  # row-halves of 4096; 8 chunks of 512 each
N_BANKS = 6             # 16 accumulators at partition slots 0/32/64

LOG2E = 1.4426950408889634
SCHRA_A = 128.0 * LOG2E            # bf16-bit-domain scale
SCHRA_B = 127.0 * 128.0 - 7.33     # exponent bias minus mean-zero correction


def split_multi_waits(nc):
    """This walrus build's CoreV2/V3 codegen rejects any instruction carrying
    more than one sync wait command. Split extra waits onto same-engine NoOps
    inserted immediately before the offending instruction (sequential waits on
    one engine are equivalent to one AND-ed wait set)."""
    n_split = 0
    for func in nc.m.functions:
        for block in func.blocks:
            new_insts = []
            for inst in block.instructions:
                si = inst.sync_info
                if si is not None and len(si.on_wait) > 1:
                    waits = list(si.on_wait)
                    for w in waits[:-1]:
                        nop = mybir.InstNoOp(
                            name=f"I-waitsplit-{nc.next_id()}",
                            sync_info=mybir.SyncInfo(on_wait=[w], on_update=[]),
                            bass_nofuse=True,
                            engine=inst.engine,
                        )
                        nc.register_instruction(nop)
                        new_insts.append(nop)
                        n_split += 1
                    si.on_wait = [waits[-1]]
                new_insts.append(inst)
            block.instructions[:] = new_insts
    return n_split


def _slot(i):
    """Accumulator i (= rh*RC + rc) -> (bank, partition slot)."""
    return i // 3, 32 * (i % 3)


def build_nc(repeat=1, v_act=V_ACT, vd_c=VD_C, xa_bufs=3, xd_bufs=3):
    """Build the single-core Bass program (same program runs SPMD on all cores).

    repeat>1 re-runs the streaming phase (identical work/results) so one
    dispatch holds R x the device work - used only for timing measurements.
    """
    n_vt = vd_c // P
    act_sub = v_act // 2
    nc = bass.Bass()
    xa = nc.dram_tensor("xa", [B_LOC, v_act], mybir.dt.bfloat16, kind="ExternalInput")
    xd = nc.dram_tensor("xd", [vd_c, B], mybir.dt.float8e4, kind="ExternalInput")
    out_a = nc.dram_tensor("out_a", [P, N_RT * 2], mybir.dt.float32, kind="ExternalOutput")
    out_d = nc.dram_tensor("out_d", [P, N_BANKS * MM], mybir.dt.float32, kind="ExternalOutput")

    with tile.TileContext(nc) as tc:
        with (
            tc.tile_pool(name="xa_p", bufs=xa_bufs) as xa_p,
            tc.tile_pool(name="xd_p", bufs=xd_bufs) as xd_p,
            tc.tile_pool(name="xb_p", bufs=2) as xb_p,
            tc.tile_pool(name="y_p", bufs=2) as y_p,
            tc.tile_pool(name="e_p", bufs=2) as e_p,
            tc.tile_pool(name="ps", bufs=1, space="PSUM") as ps,
            tc.tile_pool(name="small", bufs=1) as small,
        ):
            ones_t = small.tile([P, 1], mybir.dt.bfloat16)
            nc.gpsimd.memset(ones_t[:], 1.0)
            banks = [
                ps.tile([P, MM], mybir.dt.float32, name=f"bank{i}")
                for i in range(N_BANKS)
            ]
            partials_a = small.tile([P, N_RT * 2], mybir.dt.float32)
            res_t = small.tile([P, N_BANKS * MM], mybir.dt.float32)

            def xd_tile(vt):
                xd_t = xd_p.tile([P, B], mybir.dt.float8e4, tag="xd")
                nc.sync.dma_start(out=xd_t[:], in_=xd[vt * P : (vt + 1) * P, :])
                for rh in range(RH):
                    xb_t = xb_p.tile([P, B // RH], mybir.dt.bfloat16, tag="xb")
                    nc.vector.tensor_copy(
                        out=xb_t[:], in_=xd_t[:, rh * (B // RH) : (rh + 1) * (B // RH)]
                    )
                    y_t = y_p.tile([P, B // RH], mybir.dt.int16, tag="y")
                    nc.vector.tensor_scalar(
                        out=y_t[:], in0=xb_t[:], scalar1=SCHRA_A, scalar2=SCHRA_B,
                        op0=mybir.AluOpType.mult, op1=mybir.AluOpType.add,
                    )
                    e_bf = y_t[:].bitcast(mybir.dt.bfloat16)
                    for rc in range(RC):
                        b, slot = _slot(rh * RC + rc)
                        nc.tensor.matmul(
                            banks[b][slot : slot + 1, :],
                            ones_t[:],
                            e_bf[:, rc * MM : (rc + 1) * MM],
                            start=(vt == 0),
                            stop=(vt == n_vt - 1),
                        )

            for _rep in range(repeat):
                xd_i = 0
                for rt in range(N_RT):
                    for half in range(2):
                        xa_t = xa_p.tile([P, act_sub], mybir.dt.bfloat16, tag="xa")
                        nc.sync.dma_start(
                            out=xa_t[:],
                            in_=xa[rt * P : (rt + 1) * P,
                                   half * act_sub : (half + 1) * act_sub],
                        )
                        e_t = e_p.tile([P, act_sub], mybir.dt.bfloat16, tag="e")
                        nc.scalar.activation(
                            out=e_t[:],
                            in_=xa_t[:],
                            func=mybir.ActivationFunctionType.Exp,
                            accum_out=partials_a[:, rt * 2 + half : rt * 2 + half + 1],
                        )
                    while xd_i < (rt + 1) * n_vt // N_RT:
                        xd_tile(xd_i)
                        xd_i += 1
                for b in range(N_BANKS):
                    nc.vector.tensor_copy(
                        out=res_t[:, b * MM : (b + 1) * MM], in_=banks[b][:]
                    )
                nc.sync.dma_start(out=out_d[:], in_=res_t[:])
                nc.sync.dma_start(out=out_a[:], in_=partials_a[:])

    split_multi_waits(nc)
    return nc


def make_in_maps(output, label=None, n_cores=N_CORES):
    """Shard full inputs into per-core input maps.

    ACT share: rows batch-sharded, bf16. DVE share: columns [V_ACT, V)
    vocab-sharded across cores, transposed to [vd, rows], fp8.
    """
    import ml_dtypes

    output = np.asarray(output)
    xd_all = np.ascontiguousarray(
        output[:, V_ACT:].astype(ml_dtypes.float8_e4m3fn).T
    )  # [N_CORES*VD_C, B]
    in_maps = []
    for c in range(n_cores):
        xa_c = np.ascontiguousarray(
            output[c * B_LOC : (c + 1) * B_LOC, :V_ACT].astype(ml_dtypes.bfloat16)
        )
        xd_c = np.ascontiguousarray(xd_all[c * VD_C : (c + 1) * VD_C])
        in_maps.append({"xa": xa_c, "xd": xd_c})
    return in_maps


def combine(results, output, label):
    """Host epilogue: assemble per-row sums, then log / gather / mean."""
    sums = np.zeros(B, dtype=np.float64)
    for c, r in enumerate(results):
        pa = r["out_a"].astype(np.float64)  # [128, 16]
        for rt in range(N_RT):
            rows = slice(c * B_LOC + rt * P, c * B_LOC + (rt + 1) * P)
            sums[rows] += pa[:, rt * 2] + pa[:, rt * 2 + 1]
        pd = r["out_d"].astype(np.float64)  # [128, N_BANKS*512]
        for i in range(RH * RC):
            b, slot = _slot(i)
            rows = slice(i * MM, (i + 1) * MM)
            sums[rows] += pd[slot, b * MM : (b + 1) * MM]
    label = np.asarray(label).astype(np.int64)
    gathered = np.take_along_axis(
        np.asarray(output), label[:, None], axis=1
    )[:, 0].astype(np.float64)
    loss = (np.log(sums + EPS).sum() - gathered.sum()) / B
    return np.float32(loss)


_NC_CACHE = {}


def kernel(output, label):
    if "nc" not in _NC_CACHE:
        _NC_CACHE["nc"] = build_nc()
    nc = _NC_CACHE["nc"]
    in_maps = make_in_maps(output)
    res = run_bass_kernel_spmd(nc, in_maps, list(range(N_CORES)))
    return combine(res.results, output, label)
